# revision 15
# baseline (speedup 1.0000x reference)
"""Trainium2 Bass kernel for nn_MemoryBlock (retrieval_knn).

Data-parallel over batch: each of the 8 cores handles 4 samples and the full
(replicated) memory banks.  Per core, per level:
  - features stay SBUF-resident in fp32; an fp16 b-minor-interleaved copy
    (col = w*4 + b) serves as matmul weights so each super-chunk's stationary
    operand is a contiguous single-free-dim AP
  - memory banks stream through SBUF in fp16 tiles (cast during SWDGE DMA);
    PE accumulates the cross terms f.m for all (sample, mem) pairs as the
    diagonal blocks of a [jj*4, jj*30] PSUM tile; DVE computes per-mem sum of
    squares via fused tensor_tensor_reduce
  - argmin over the 30 combined distances via max_with_indices on the negated
    scores; the winning index drives dynamic-offset DMA gathers
  - (selected - f)^2 in exact fp32 on DVE, both output halves DMA'd to DRAM

Numerics: fp16 input rounding perturbs each distance by ~1e-5 while the
minimum argmin gap on N(0,1) data at these sizes is ~2.4e-4, so the argmin is
stable; the final outputs are computed entirely in fp32.
"""

import numpy as np

B_TOT = 32
NCORES = 8
B_LOC = B_TOT // NCORES  # 4
M = 30
P = 128
W = 256          # m-tile columns (per mem sample) per streaming tile
JJ = 16          # chunks packed per matmul (free dim = JJ*M = 480)

# per level: (C, H, W_sp)
LEVELS = [(64, 64, 64), (128, 32, 32), (256, 16, 16)]
FS = [(c * h * w) // P for (c, h, w) in LEVELS]          # [2048, 1024, 512]
NS = [c * h * w for (c, h, w) in LEVELS]                 # elements per sample

_cache = {}


def _build(stage=4, substage=3):
    import concourse.bass as bass
    import concourse.tile as tile
    import concourse.mybir as mybir
    from concourse import bacc

    nc = bacc.Bacc(
        "TRN2",
        target_bir_lowering=False,
        debug=False,
        enable_asserts=True,
        num_devices=NCORES,
    )
    fp32 = mybir.dt.float32
    fp16 = mybir.dt.float16

    f_dram = [
        nc.dram_tensor(f"f{l}", [B_LOC, P, FS[l]], fp32, kind="ExternalInput").ap()
        for l in range(3)
    ]
    m_dram = [
        nc.dram_tensor(f"mem{l}", [M, P, FS[l]], fp32, kind="ExternalInput").ap()
        for l in range(3)
    ]
    out_dram = [
        nc.dram_tensor(f"out{l}", [B_LOC, 2, P, FS[l]], fp32, kind="ExternalOutput").ap()
        for l in range(3)
    ]
    dbg = nc.dram_tensor("dbg", [B_LOC, M], fp32, kind="ExternalOutput").ap()

    with tile.TileContext(nc) as tc:
        with (
            tc.tile_pool(name="fres", bufs=1) as fpool,
            tc.tile_pool(name="mstream", bufs=4) as mpool,
            tc.tile_pool(name="scratch", bufs=1) as spool,
            tc.tile_pool(name="small", bufs=1) as small,
            tc.tile_pool(name="psum", bufs=2, space="PSUM") as psp,
            tc.tile_pool(name="sel", bufs=2) as selpool,
            tc.tile_pool(name="dsq", bufs=2) as dpool,
        ):
            # ---- resident features: fp32 master + fp16 interleaved weights ----
            f_sb = []
            f_w = []
            for l in range(3):
                t = fpool.tile([P, B_LOC * FS[l]], fp32, tag=f"f{l}")
                nc.sync.dma_start(
                    t[:].rearrange("p (b f) -> p b f", b=B_LOC),
                    f_dram[l].rearrange("b p f -> p b f"),
                )
                f_sb.append(t)
                w = fpool.tile([P, FS[l] * B_LOC], fp16, tag=f"fw{l}")
                # col = w*4 + b  (b-minor interleave): (jj,b) weight slices are
                # contiguous single-dim APs
                nc.vector.tensor_copy(
                    w[:].rearrange("p (f b) -> p b f", b=B_LOC),
                    t[:].rearrange("p (b f) -> p b f", b=B_LOC),
                )
                f_w.append(w)

            # f-half of every output does not depend on anything else: copy out now
            for l in range(3):
                for b in range(B_LOC):
                    nc.sync.dma_start(
                        out_dram[l][b, 0],
                        f_sb[l][:, b * FS[l]:(b + 1) * FS[l]],
                    )

            ones_sb = small.tile([P, B_LOC], fp16)
            nc.vector.memset(ones_sb[:], 1.0)
            acc_s = small.tile([B_LOC, M], fp32)
            nc.vector.memset(acc_s[:], 0.0)

            # consume-tile for DMA-only substages (prevents DCE of m loads)
            consume = small.tile([32, M], fp32)
            # ---- phase 1: stream memory banks, accumulate cross + msq ----
            for l in range(3 if stage >= 2 else 0):
                F = FS[l]
                T = F // W
                n_sc = W // JJ
                m_v3 = m_dram[l].rearrange("k p f -> p k f")

                ps = psp.tile([JJ * B_LOC, JJ * M], fp32, tag="cross")
                ps3 = ps[:].rearrange("a (j k) -> a j k", j=JJ)
                ps_msq = psp.tile([B_LOC, JJ * M], fp32, tag="msq")
                ps_msq3 = ps_msq[:].rearrange("a (j k) -> a j k", j=JJ)

                for t in range(T):
                    mt = mpool.tile([P, M * W], fp16, tag="mt")
                    nc.gpsimd.dma_start(
                        mt[:].rearrange("p (k w) -> p k w", k=M),
                        m_v3[:, :, t * W:(t + 1) * W],
                    )
                    mtv = mt[:].rearrange("p (k w) -> p k w", k=M)
                    if substage == 1:
                        nc.vector.tensor_copy(consume[:], mt[0:32, 0:M])
                        continue
                    for sc in range(n_sc):
                        lhsT = f_w[l][:, (t * n_sc + sc) * JJ * B_LOC:
                                      (t * n_sc + sc + 1) * JJ * B_LOC]
                        rhs = mtv[:, :, sc * JJ:(sc + 1) * JJ].rearrange("p k j -> p j k")
                        nc.tensor.matmul(
                            ps3, lhsT, rhs,
                            start=(t == 0 and sc == 0),
                            stop=(t == T - 1 and sc == n_sc - 1),
                        )
                    if substage >= 3:
                        # msq: square the tile (split ACT/DVE to halve the
                        # critical path), partition-reduce with a ones-weight
                        # matmul into the (jj,k) msq accumulator
                        sq_full = spool.tile([P, M * W], fp16, tag="sqfull")
                        half = 10 * W  # 10 mems squared on ACT, 20 on DVE
                        nc.scalar.square(sq_full[:, :half], mt[:, :half])
                        nc.vector.tensor_tensor(
                            sq_full[:, half:], mt[:, half:], mt[:, half:],
                            mybir.AluOpType.mult)
                        sqv = sq_full[:].rearrange("p (k w) -> p k w", k=M)
                        for sc in range(n_sc):
                            rhs = sqv[:, :, sc * JJ:(sc + 1) * JJ].rearrange("p k j -> p j k")
                            nc.tensor.matmul(
                                ps_msq3, ones_sb[:], rhs,
                                start=(t == 0 and sc == 0),
                                stop=(t == T - 1 and sc == n_sc - 1),
                            )

                # ---- level epilogue: fold psum diag + msq into acc_s ----
                if substage == 1:
                    continue
                # engine APs need 32-aligned partition bases, so route the
                # diagonal blocks through a full DVE copy + per-block DMAs
                cp = spool.tile([JJ * B_LOC, JJ * M], fp32, tag="pscp")
                nc.vector.tensor_copy(cp[:], ps[:])
                diag = spool.tile([B_LOC, JJ * M], fp32, tag=f"diag{l}")
                for jj in range(JJ):
                    nc.sync.dma_start(
                        diag[:, jj * M:(jj + 1) * M],
                        cp[jj * B_LOC:(jj + 1) * B_LOC, jj * M:(jj + 1) * M],
                    )
                lvl = small.tile([B_LOC, M], fp32, tag=f"lvl{l}")
                nc.vector.tensor_reduce(
                    lvl[:],
                    diag[:].rearrange("p (j k) -> p k j", j=JJ),
                    mybir.AxisListType.X,
                    mybir.AluOpType.add,
                )
                if substage >= 3:
                    msq_red = small.tile([B_LOC, M], fp32, tag=f"msqr{l}")
                    nc.vector.tensor_reduce(
                        msq_red[:],
                        ps_msq[:].rearrange("p (j k) -> p k j", j=JJ),
                        mybir.AxisListType.X,
                        mybir.AluOpType.add,
                    )
                inv_n = 1.0 / float(NS[l])
                nc.vector.tensor_scalar(
                    lvl[:], lvl[:], -2.0 * inv_n, None, mybir.AluOpType.mult)
                if substage >= 3:
                    nc.vector.tensor_scalar(
                        msq_red[:], msq_red[:], inv_n, None, mybir.AluOpType.mult)
                    nc.vector.tensor_tensor(lvl[:], lvl[:], msq_red[:], mybir.AluOpType.add)
                nc.vector.tensor_tensor(acc_s[:], acc_s[:], lvl[:], mybir.AluOpType.add)

            if stage >= 2:
                nc.sync.dma_start(dbg[:], acc_s[:])
            # ---- argmin ----
            neg = small.tile([B_LOC, M], fp32)
            if stage >= 3:
                nc.vector.tensor_scalar(neg[:], acc_s[:], -1.0, None, mybir.AluOpType.mult)
                mx = small.tile([B_LOC, 8], fp32)
                mi = small.tile([B_LOC, 8], mybir.dt.uint32)
                nc.vector.max_with_indices(mx[:], mi[:], neg[:])
                # fold the per-sample winners (column 0 of each partition) into
                # one partition-0 row, then expand to per-partition offsets
                mi_t = small.tile([1, B_LOC], mybir.dt.uint32)
                nc.sync.dma_start(mi_t[0:1, :], mi[0:B_LOC, 0:1])
                iota_t = small.tile([P, 1], mybir.dt.uint32)
                nc.gpsimd.iota(iota_t[:], pattern=[[0, 1]], base=0, channel_multiplier=1)

            # ---- phase 2: gather selected mem, (sel-f)^2, write out ----
            for b in range(B_LOC if stage >= 4 else 0):
                # offsets[p] = idx_b * 128 + p, rows of the [M*128, F] views
                idxb = small.tile([P, 1], mybir.dt.uint32, tag=f"idxb{b}")
                nc.gpsimd.partition_broadcast(idxb[:], mi_t[0:1, b:b + 1])
                offs = small.tile([P, 1], mybir.dt.uint32, tag=f"offs{b}")
                nc.vector.tensor_scalar(offs[:], idxb[:], P, None, mybir.AluOpType.mult)
                nc.vector.tensor_tensor(offs[:], offs[:], iota_t[:], mybir.AluOpType.add)
                for l in range(3):
                    F = FS[l]
                    sel = selpool.tile([P, F], fp32, tag="sel")
                    nc.gpsimd.indirect_dma_start(
                        out=sel[:],
                        out_offset=None,
                        in_=m_dram[l].rearrange("k p f -> (k p) f"),
                        in_offset=bass.IndirectOffsetOnAxis(ap=offs[:, 0:1], axis=0),
                    )
                    d = dpool.tile([P, F], fp32, tag="d")
                    nc.vector.tensor_tensor(
                        d[:], sel[:],
                        f_sb[l][:, b * F:(b + 1) * F],
                        mybir.AluOpType.subtract,
                    )
                    nc.vector.tensor_tensor(d[:], d[:], d[:], mybir.AluOpType.mult)
                    nc.sync.dma_start(out_dram[l][b, 1], d[:])

    nc.compile()
    return nc


def _get_nc():
    if "nc" not in _cache:
        _cache["nc"] = _build()
    return _cache["nc"]


def kernel(f0, f1, f2, mem0, mem1, mem2, **kwargs):
    from concourse.bass_utils import run_bass_kernel_spmd

    nc = _get_nc()
    feats = [np.ascontiguousarray(x, dtype=np.float32).reshape(B_TOT, P, FS[l])
             for l, x in enumerate((f0, f1, f2))]
    mems = [np.ascontiguousarray(x, dtype=np.float32).reshape(M, P, FS[l])
            for l, x in enumerate((mem0, mem1, mem2))]

    in_maps = []
    for i in range(NCORES):
        sl = slice(i * B_LOC, (i + 1) * B_LOC)
        im = {f"f{l}": feats[l][sl] for l in range(3)}
        im.update({f"mem{l}": mems[l] for l in range(3)})
        in_maps.append(im)

    res = run_bass_kernel_spmd(nc, in_maps, core_ids=list(range(NCORES)))
    kernel._last_results = res

    outs = []
    for l, (c, h, w) in enumerate(LEVELS):
        full = np.concatenate([res.results[i][f"out{l}"] for i in range(NCORES)], axis=0)
        outs.append(full.reshape(B_TOT, 2 * c, h, w))
    return tuple(outs)


# revision 16
# speedup vs baseline: 1.0062x; 1.0062x over previous
"""Trainium2 Bass kernel for nn_MemoryBlock (retrieval_knn).

Data-parallel over batch: each of the 8 cores handles 4 samples and the full
(replicated) memory banks.  Per core, per level:
  - features stay SBUF-resident in fp32; an fp16 b-minor-interleaved copy
    (col = w*4 + b) serves as matmul weights so each super-chunk's stationary
    operand is a contiguous single-free-dim AP
  - memory banks stream through SBUF in fp16 tiles (cast during SWDGE DMA);
    PE accumulates the cross terms f.m for all (sample, mem) pairs as the
    diagonal blocks of a [jj*4, jj*30] PSUM tile; DVE computes per-mem sum of
    squares via fused tensor_tensor_reduce
  - argmin over the 30 combined distances via max_with_indices on the negated
    scores; the winning index drives dynamic-offset DMA gathers
  - (selected - f)^2 in exact fp32 on DVE, both output halves DMA'd to DRAM

Numerics: fp16 input rounding perturbs each distance by ~1e-5 while the
minimum argmin gap on N(0,1) data at these sizes is ~2.4e-4, so the argmin is
stable; the final outputs are computed entirely in fp32.
"""

import numpy as np

B_TOT = 32
NCORES = 8
B_LOC = B_TOT // NCORES  # 4
M = 30
P = 128
W = 256          # m-tile columns (per mem sample) per streaming tile
JJ = 16          # chunks packed per matmul (free dim = JJ*M = 480)

# per level: (C, H, W_sp)
LEVELS = [(64, 64, 64), (128, 32, 32), (256, 16, 16)]
FS = [(c * h * w) // P for (c, h, w) in LEVELS]          # [2048, 1024, 512]
NS = [c * h * w for (c, h, w) in LEVELS]                 # elements per sample

_cache = {}


def _build(stage=4, substage=3):
    import concourse.bass as bass
    import concourse.tile as tile
    import concourse.mybir as mybir
    from concourse import bacc

    nc = bacc.Bacc(
        "TRN2",
        target_bir_lowering=False,
        debug=False,
        enable_asserts=True,
        num_devices=NCORES,
    )
    fp32 = mybir.dt.float32
    fp16 = mybir.dt.float16

    f_dram = [
        nc.dram_tensor(f"f{l}", [B_LOC, P, FS[l]], fp32, kind="ExternalInput").ap()
        for l in range(3)
    ]
    m_dram = [
        nc.dram_tensor(f"mem{l}", [M, P, FS[l]], fp32, kind="ExternalInput").ap()
        for l in range(3)
    ]
    out_dram = [
        nc.dram_tensor(f"out{l}", [B_LOC, 2, P, FS[l]], fp32, kind="ExternalOutput").ap()
        for l in range(3)
    ]
    dbg = nc.dram_tensor("dbg", [B_LOC, M], fp32, kind="ExternalOutput").ap()

    with tile.TileContext(nc) as tc:
        with (
            tc.tile_pool(name="fres", bufs=1) as fpool,
            tc.tile_pool(name="mstream", bufs=4) as mpool,
            tc.tile_pool(name="scratch", bufs=1) as spool,
            tc.tile_pool(name="small", bufs=1) as small,
            tc.tile_pool(name="psum", bufs=2, space="PSUM") as psp,
            tc.tile_pool(name="sel", bufs=2) as selpool,
            tc.tile_pool(name="dsq", bufs=2) as dpool,
        ):
            # ---- resident features: fp32 master + fp16 interleaved weights ----
            f_sb = []
            f_w = []
            for l in range(3):
                t = fpool.tile([P, B_LOC * FS[l]], fp32, tag=f"f{l}")
                nc.sync.dma_start(
                    t[:].rearrange("p (b f) -> p b f", b=B_LOC),
                    f_dram[l].rearrange("b p f -> p b f"),
                )
                f_sb.append(t)
                w = fpool.tile([P, FS[l] * B_LOC], fp16, tag=f"fw{l}")
                # col = w*4 + b  (b-minor interleave): (jj,b) weight slices are
                # contiguous single-dim APs
                nc.vector.tensor_copy(
                    w[:].rearrange("p (f b) -> p b f", b=B_LOC),
                    t[:].rearrange("p (b f) -> p b f", b=B_LOC),
                )
                f_w.append(w)

            # f-half of every output does not depend on anything else: copy out now
            for l in range(3):
                for b in range(B_LOC):
                    nc.sync.dma_start(
                        out_dram[l][b, 0],
                        f_sb[l][:, b * FS[l]:(b + 1) * FS[l]],
                    )

            ones_sb = small.tile([P, B_LOC], fp16)
            nc.vector.memset(ones_sb[:], 1.0)
            acc_s = small.tile([B_LOC, M], fp32)
            nc.vector.memset(acc_s[:], 0.0)

            # consume-tile for DMA-only substages (prevents DCE of m loads)
            consume = small.tile([32, M], fp32)
            # ---- phase 1: stream memory banks, accumulate cross + msq ----
            for l in range(3 if stage >= 2 else 0):
                F = FS[l]
                T = F // W
                n_sc = W // JJ
                m_v3 = m_dram[l].rearrange("k p f -> p k f")

                ps = psp.tile([JJ * B_LOC, JJ * M], fp32, tag="cross")
                ps3 = ps[:].rearrange("a (j k) -> a j k", j=JJ)
                ps_msq = psp.tile([B_LOC, JJ * M], fp32, tag="msq")
                ps_msq3 = ps_msq[:].rearrange("a (j k) -> a j k", j=JJ)

                for t in range(T):
                    mt = mpool.tile([P, M * W], fp16, tag="mt")
                    nc.gpsimd.dma_start(
                        mt[:].rearrange("p (k w) -> p k w", k=M),
                        m_v3[:, :, t * W:(t + 1) * W],
                    )
                    mtv = mt[:].rearrange("p (k w) -> p k w", k=M)
                    if substage == 1:
                        nc.vector.tensor_copy(consume[:], mt[0:32, 0:M])
                        continue
                    for sc in range(n_sc):
                        lhsT = f_w[l][:, (t * n_sc + sc) * JJ * B_LOC:
                                      (t * n_sc + sc + 1) * JJ * B_LOC]
                        rhs = mtv[:, :, sc * JJ:(sc + 1) * JJ].rearrange("p k j -> p j k")
                        nc.tensor.matmul(
                            ps3, lhsT, rhs,
                            start=(t == 0 and sc == 0),
                            stop=(t == T - 1 and sc == n_sc - 1),
                        )
                    if substage >= 3:
                        # msq: square the tile (split ACT/DVE to halve the
                        # critical path), partition-reduce with a ones-weight
                        # matmul into the (jj,k) msq accumulator
                        sq_full = spool.tile([P, M * W], fp16, tag="sqfull")
                        half = 10 * W  # 10 mems squared on ACT, 20 on DVE
                        nc.scalar.square(sq_full[:, :half], mt[:, :half])
                        nc.vector.tensor_tensor(
                            sq_full[:, half:], mt[:, half:], mt[:, half:],
                            mybir.AluOpType.mult)
                        sqv = sq_full[:].rearrange("p (k w) -> p k w", k=M)
                        for sc in range(n_sc):
                            rhs = sqv[:, :, sc * JJ:(sc + 1) * JJ].rearrange("p k j -> p j k")
                            nc.tensor.matmul(
                                ps_msq3, ones_sb[:], rhs,
                                start=(t == 0 and sc == 0),
                                stop=(t == T - 1 and sc == n_sc - 1),
                            )

                # ---- level epilogue: fold psum diag + msq into acc_s ----
                if substage == 1:
                    continue
                # engine APs need 32-aligned partition bases, so route the
                # diagonal blocks through a full DVE copy + per-block DMAs
                cp = spool.tile([JJ * B_LOC, JJ * M], fp32, tag="pscp")
                nc.vector.tensor_copy(cp[:], ps[:])
                diag = spool.tile([B_LOC, JJ * M], fp32, tag=f"diag{l}")
                for jj in range(JJ):
                    nc.sync.dma_start(
                        diag[:, jj * M:(jj + 1) * M],
                        cp[jj * B_LOC:(jj + 1) * B_LOC, jj * M:(jj + 1) * M],
                    )
                lvl = small.tile([B_LOC, M], fp32, tag=f"lvl{l}")
                nc.vector.tensor_reduce(
                    lvl[:],
                    diag[:].rearrange("p (j k) -> p k j", j=JJ),
                    mybir.AxisListType.X,
                    mybir.AluOpType.add,
                )
                if substage >= 3:
                    msq_red = small.tile([B_LOC, M], fp32, tag=f"msqr{l}")
                    nc.vector.tensor_reduce(
                        msq_red[:],
                        ps_msq[:].rearrange("p (j k) -> p k j", j=JJ),
                        mybir.AxisListType.X,
                        mybir.AluOpType.add,
                    )
                inv_n = 1.0 / float(NS[l])
                nc.vector.tensor_scalar(
                    lvl[:], lvl[:], -2.0 * inv_n, None, mybir.AluOpType.mult)
                if substage >= 3:
                    nc.vector.tensor_scalar(
                        msq_red[:], msq_red[:], inv_n, None, mybir.AluOpType.mult)
                    nc.vector.tensor_tensor(lvl[:], lvl[:], msq_red[:], mybir.AluOpType.add)
                nc.vector.tensor_tensor(acc_s[:], acc_s[:], lvl[:], mybir.AluOpType.add)

            if stage >= 2:
                nc.sync.dma_start(dbg[:], acc_s[:])
            # ---- argmin ----
            neg = small.tile([B_LOC, M], fp32)
            if stage >= 3:
                nc.vector.tensor_scalar(neg[:], acc_s[:], -1.0, None, mybir.AluOpType.mult)
                mx = small.tile([B_LOC, 8], fp32)
                mi = small.tile([B_LOC, 8], mybir.dt.uint32)
                nc.vector.max_with_indices(mx[:], mi[:], neg[:])
                # fold the per-sample winners (column 0 of each partition) into
                # one partition-0 row, then expand to per-partition offsets
                mi_t = small.tile([1, B_LOC], mybir.dt.uint32)
                nc.sync.dma_start(mi_t[0:1, :], mi[0:B_LOC, 0:1])
                iota_t = small.tile([P, 1], mybir.dt.uint32)
                nc.gpsimd.iota(iota_t[:], pattern=[[0, 1]], base=0, channel_multiplier=1)

            # ---- phase 2: gather selected mem, (sel-f)^2, write out ----
            for b in range(B_LOC if stage >= 4 else 0):
                # offsets[p] = idx_b * 128 + p, rows of the [M*128, F] views
                idxb = small.tile([P, 1], mybir.dt.uint32, tag=f"idxb{b}")
                nc.gpsimd.partition_broadcast(idxb[:], mi_t[0:1, b:b + 1])
                offs = small.tile([P, 1], mybir.dt.uint32, tag=f"offs{b}")
                nc.vector.tensor_scalar(offs[:], idxb[:], P, None, mybir.AluOpType.mult)
                nc.vector.tensor_tensor(offs[:], offs[:], iota_t[:], mybir.AluOpType.add)
                for l in range(3):
                    F = FS[l]
                    sel = selpool.tile([P, F], fp32, tag="sel")
                    nc.gpsimd.indirect_dma_start(
                        out=sel[:],
                        out_offset=None,
                        in_=m_dram[l].rearrange("k p f -> (k p) f"),
                        in_offset=bass.IndirectOffsetOnAxis(ap=offs[:, 0:1], axis=0),
                    )
                    d = dpool.tile([P, F], fp32, tag="d")
                    nc.vector.tensor_tensor(
                        d[:], sel[:],
                        f_sb[l][:, b * F:(b + 1) * F],
                        mybir.AluOpType.subtract,
                    )
                    if l == 0:
                        # square on ACT into the dead gather buffer: halves the
                        # DVE-serial tail after the argmin
                        nc.scalar.square(sel[:], d[:])
                        nc.sync.dma_start(out_dram[l][b, 1], sel[:])
                    else:
                        nc.vector.tensor_tensor(d[:], d[:], d[:], mybir.AluOpType.mult)
                        nc.sync.dma_start(out_dram[l][b, 1], d[:])

    nc.compile()
    return nc


def _get_nc():
    if "nc" not in _cache:
        _cache["nc"] = _build()
    return _cache["nc"]


def kernel(f0, f1, f2, mem0, mem1, mem2, **kwargs):
    from concourse.bass_utils import run_bass_kernel_spmd

    nc = _get_nc()
    feats = [np.ascontiguousarray(x, dtype=np.float32).reshape(B_TOT, P, FS[l])
             for l, x in enumerate((f0, f1, f2))]
    mems = [np.ascontiguousarray(x, dtype=np.float32).reshape(M, P, FS[l])
            for l, x in enumerate((mem0, mem1, mem2))]

    in_maps = []
    for i in range(NCORES):
        sl = slice(i * B_LOC, (i + 1) * B_LOC)
        im = {f"f{l}": feats[l][sl] for l in range(3)}
        im.update({f"mem{l}": mems[l] for l in range(3)})
        in_maps.append(im)

    res = run_bass_kernel_spmd(nc, in_maps, core_ids=list(range(NCORES)))
    kernel._last_results = res

    outs = []
    for l, (c, h, w) in enumerate(LEVELS):
        full = np.concatenate([res.results[i][f"out{l}"] for i in range(NCORES)], axis=0)
        outs.append(full.reshape(B_TOT, 2 * c, h, w))
    return tuple(outs)


# revision 18
# speedup vs baseline: 1.0956x; 1.0888x over previous
"""Trainium2 Bass kernel for nn_MemoryBlock (retrieval_knn).

Data-parallel over batch: each of the 8 cores handles 4 samples and the full
(replicated) memory banks.  Per core, per level:
  - features stay SBUF-resident in fp32; an fp16 b-minor-interleaved copy
    (col = w*4 + b) serves as matmul weights so each super-chunk's stationary
    operand is a contiguous single-free-dim AP
  - memory banks stream through SBUF in fp16 tiles (cast during SWDGE DMA);
    PE accumulates the cross terms f.m for all (sample, mem) pairs as the
    diagonal blocks of a [jj*4, jj*30] PSUM tile; DVE computes per-mem sum of
    squares via fused tensor_tensor_reduce
  - argmin over the 30 combined distances via max_with_indices on the negated
    scores; the winning index drives dynamic-offset DMA gathers
  - (selected - f)^2 in exact fp32 on DVE, both output halves DMA'd to DRAM

Numerics: fp16 input rounding perturbs each distance by ~1e-5 while the
minimum argmin gap on N(0,1) data at these sizes is ~2.4e-4, so the argmin is
stable; the final outputs are computed entirely in fp32.
"""

import numpy as np

B_TOT = 32
NCORES = 8
B_LOC = B_TOT // NCORES  # 4
M = 30
P = 128
W = 256          # m-tile columns (per mem sample) per streaming tile
JJ = 16          # chunks packed per matmul (free dim = JJ*M = 480)

# per level: (C, H, W_sp)
LEVELS = [(64, 64, 64), (128, 32, 32), (256, 16, 16)]
FS = [(c * h * w) // P for (c, h, w) in LEVELS]          # [2048, 1024, 512]
NS = [c * h * w for (c, h, w) in LEVELS]                 # elements per sample

_cache = {}


def _build(stage=4, substage=3, MB=3, SB=3, DB=3):
    import concourse.bass as bass
    import concourse.tile as tile
    import concourse.mybir as mybir
    from concourse import bacc

    nc = bacc.Bacc(
        "TRN2",
        target_bir_lowering=False,
        debug=False,
        enable_asserts=True,
        num_devices=NCORES,
    )
    fp32 = mybir.dt.float32
    fp16 = mybir.dt.float16

    f_dram = [
        nc.dram_tensor(f"f{l}", [B_LOC, P, FS[l]], fp32, kind="ExternalInput").ap()
        for l in range(3)
    ]
    m_dram = [
        nc.dram_tensor(f"mem{l}", [M, P, FS[l]], fp32, kind="ExternalInput").ap()
        for l in range(3)
    ]
    out_dram = [
        nc.dram_tensor(f"out{l}", [B_LOC, 2, P, FS[l]], fp32, kind="ExternalOutput").ap()
        for l in range(3)
    ]
    dbg = nc.dram_tensor("dbg", [B_LOC, M], fp32, kind="ExternalOutput").ap()

    with tile.TileContext(nc) as tc:
        with (
            tc.tile_pool(name="fres", bufs=1) as fpool,
            tc.tile_pool(name="mstream", bufs=MB) as mpool,
            tc.tile_pool(name="scratch", bufs=1) as spool,
            tc.tile_pool(name="small", bufs=1) as small,
            tc.tile_pool(name="psum", bufs=2, space="PSUM") as psp,
            tc.tile_pool(name="sel", bufs=SB) as selpool,
            tc.tile_pool(name="dsq", bufs=DB) as dpool,
        ):
            # ---- resident features: fp32 master + fp16 interleaved weights ----
            f_sb = []
            f_w = []
            for l in range(3):
                t = fpool.tile([P, B_LOC * FS[l]], fp32, tag=f"f{l}")
                nc.sync.dma_start(
                    t[:].rearrange("p (b f) -> p b f", b=B_LOC),
                    f_dram[l].rearrange("b p f -> p b f"),
                )
                f_sb.append(t)
                w = fpool.tile([P, FS[l] * B_LOC], fp16, tag=f"fw{l}")
                # col = w*4 + b  (b-minor interleave): (jj,b) weight slices are
                # contiguous single-dim APs
                nc.vector.tensor_copy(
                    w[:].rearrange("p (f b) -> p b f", b=B_LOC),
                    t[:].rearrange("p (b f) -> p b f", b=B_LOC),
                )
                f_w.append(w)

            # f-half of every output does not depend on anything else: copy out now
            for l in range(3):
                for b in range(B_LOC):
                    nc.sync.dma_start(
                        out_dram[l][b, 0],
                        f_sb[l][:, b * FS[l]:(b + 1) * FS[l]],
                    )

            ones_sb = small.tile([P, B_LOC], fp16)
            nc.vector.memset(ones_sb[:], 1.0)
            acc_s = small.tile([B_LOC, M], fp32)
            nc.vector.memset(acc_s[:], 0.0)

            # consume-tile for DMA-only substages (prevents DCE of m loads)
            consume = small.tile([32, M], fp32)
            # ---- phase 1: stream memory banks, accumulate cross + msq ----
            for l in range(3 if stage >= 2 else 0):
                F = FS[l]
                T = F // W
                n_sc = W // JJ
                m_v3 = m_dram[l].rearrange("k p f -> p k f")

                ps = psp.tile([JJ * B_LOC, JJ * M], fp32, tag="cross")
                ps3 = ps[:].rearrange("a (j k) -> a j k", j=JJ)
                ps_msq = psp.tile([B_LOC, JJ * M], fp32, tag="msq")
                ps_msq3 = ps_msq[:].rearrange("a (j k) -> a j k", j=JJ)

                for t in range(T):
                    mt = mpool.tile([P, M * W], fp16, tag="mt")
                    nc.gpsimd.dma_start(
                        mt[:].rearrange("p (k w) -> p k w", k=M),
                        m_v3[:, :, t * W:(t + 1) * W],
                    )
                    mtv = mt[:].rearrange("p (k w) -> p k w", k=M)
                    if substage == 1:
                        nc.vector.tensor_copy(consume[:], mt[0:32, 0:M])
                        continue
                    for sc in range(n_sc):
                        lhsT = f_w[l][:, (t * n_sc + sc) * JJ * B_LOC:
                                      (t * n_sc + sc + 1) * JJ * B_LOC]
                        rhs = mtv[:, :, sc * JJ:(sc + 1) * JJ].rearrange("p k j -> p j k")
                        nc.tensor.matmul(
                            ps3, lhsT, rhs,
                            start=(t == 0 and sc == 0),
                            stop=(t == T - 1 and sc == n_sc - 1),
                        )
                    if substage >= 3:
                        # msq: square the tile (split ACT/DVE to halve the
                        # critical path), partition-reduce with a ones-weight
                        # matmul into the (jj,k) msq accumulator
                        sq_full = spool.tile([P, M * W], fp16, tag="sqfull")
                        half = 10 * W  # 10 mems squared on ACT, 20 on DVE
                        nc.scalar.square(sq_full[:, :half], mt[:, :half])
                        nc.vector.tensor_tensor(
                            sq_full[:, half:], mt[:, half:], mt[:, half:],
                            mybir.AluOpType.mult)
                        sqv = sq_full[:].rearrange("p (k w) -> p k w", k=M)
                        for sc in range(n_sc):
                            rhs = sqv[:, :, sc * JJ:(sc + 1) * JJ].rearrange("p k j -> p j k")
                            nc.tensor.matmul(
                                ps_msq3, ones_sb[:], rhs,
                                start=(t == 0 and sc == 0),
                                stop=(t == T - 1 and sc == n_sc - 1),
                            )

                # ---- level epilogue: fold psum diag + msq into acc_s ----
                if substage == 1:
                    continue
                # engine APs need 32-aligned partition bases, so route the
                # diagonal blocks through a full DVE copy + per-block DMAs
                cp = spool.tile([JJ * B_LOC, JJ * M], fp32, tag="pscp")
                nc.vector.tensor_copy(cp[:], ps[:])
                diag = spool.tile([B_LOC, JJ * M], fp32, tag=f"diag{l}")
                for jj in range(JJ):
                    nc.sync.dma_start(
                        diag[:, jj * M:(jj + 1) * M],
                        cp[jj * B_LOC:(jj + 1) * B_LOC, jj * M:(jj + 1) * M],
                    )
                lvl = small.tile([B_LOC, M], fp32, tag=f"lvl{l}")
                nc.vector.tensor_reduce(
                    lvl[:],
                    diag[:].rearrange("p (j k) -> p k j", j=JJ),
                    mybir.AxisListType.X,
                    mybir.AluOpType.add,
                )
                if substage >= 3:
                    msq_red = small.tile([B_LOC, M], fp32, tag=f"msqr{l}")
                    nc.vector.tensor_reduce(
                        msq_red[:],
                        ps_msq[:].rearrange("p (j k) -> p k j", j=JJ),
                        mybir.AxisListType.X,
                        mybir.AluOpType.add,
                    )
                inv_n = 1.0 / float(NS[l])
                nc.vector.tensor_scalar(
                    lvl[:], lvl[:], -2.0 * inv_n, None, mybir.AluOpType.mult)
                if substage >= 3:
                    nc.vector.tensor_scalar(
                        msq_red[:], msq_red[:], inv_n, None, mybir.AluOpType.mult)
                    nc.vector.tensor_tensor(lvl[:], lvl[:], msq_red[:], mybir.AluOpType.add)
                nc.vector.tensor_tensor(acc_s[:], acc_s[:], lvl[:], mybir.AluOpType.add)

            if stage >= 2:
                nc.sync.dma_start(dbg[:], acc_s[:])
            # ---- argmin ----
            neg = small.tile([B_LOC, M], fp32)
            if stage >= 3:
                nc.vector.tensor_scalar(neg[:], acc_s[:], -1.0, None, mybir.AluOpType.mult)
                mx = small.tile([B_LOC, 8], fp32)
                mi = small.tile([B_LOC, 8], mybir.dt.uint32)
                nc.vector.max_with_indices(mx[:], mi[:], neg[:])
                # fold the per-sample winners (column 0 of each partition) into
                # one partition-0 row, then expand to per-partition offsets
                mi_t = small.tile([1, B_LOC], mybir.dt.uint32)
                nc.sync.dma_start(mi_t[0:1, :], mi[0:B_LOC, 0:1])
                iota_t = small.tile([P, 1], mybir.dt.uint32)
                nc.gpsimd.iota(iota_t[:], pattern=[[0, 1]], base=0, channel_multiplier=1)

            # ---- phase 2: gather selected mem, (sel-f)^2, write out ----
            for b in range(B_LOC if stage >= 4 else 0):
                # offsets[p] = idx_b * 128 + p, rows of the [M*128, F] views
                idxb = small.tile([P, 1], mybir.dt.uint32, tag=f"idxb{b}")
                nc.gpsimd.partition_broadcast(idxb[:], mi_t[0:1, b:b + 1])
                offs = small.tile([P, 1], mybir.dt.uint32, tag=f"offs{b}")
                nc.vector.tensor_scalar(offs[:], idxb[:], P, None, mybir.AluOpType.mult)
                nc.vector.tensor_tensor(offs[:], offs[:], iota_t[:], mybir.AluOpType.add)
                for l in range(3):
                    F = FS[l]
                    sel = selpool.tile([P, F], fp32, tag="sel")
                    nc.gpsimd.indirect_dma_start(
                        out=sel[:],
                        out_offset=None,
                        in_=m_dram[l].rearrange("k p f -> (k p) f"),
                        in_offset=bass.IndirectOffsetOnAxis(ap=offs[:, 0:1], axis=0),
                    )
                    d = dpool.tile([P, F], fp32, tag="d")
                    nc.vector.tensor_tensor(
                        d[:], sel[:],
                        f_sb[l][:, b * F:(b + 1) * F],
                        mybir.AluOpType.subtract,
                    )
                    if l == 0:
                        # square on ACT into the dead gather buffer: halves the
                        # DVE-serial tail after the argmin
                        nc.scalar.square(sel[:], d[:])
                        nc.sync.dma_start(out_dram[l][b, 1], sel[:])
                    else:
                        nc.vector.tensor_tensor(d[:], d[:], d[:], mybir.AluOpType.mult)
                        nc.sync.dma_start(out_dram[l][b, 1], d[:])

    nc.compile()
    return nc


def _get_nc():
    if "nc" not in _cache:
        _cache["nc"] = _build()
    return _cache["nc"]


def kernel(f0, f1, f2, mem0, mem1, mem2, **kwargs):
    from concourse.bass_utils import run_bass_kernel_spmd

    nc = _get_nc()
    feats = [np.ascontiguousarray(x, dtype=np.float32).reshape(B_TOT, P, FS[l])
             for l, x in enumerate((f0, f1, f2))]
    mems = [np.ascontiguousarray(x, dtype=np.float32).reshape(M, P, FS[l])
            for l, x in enumerate((mem0, mem1, mem2))]

    in_maps = []
    for i in range(NCORES):
        sl = slice(i * B_LOC, (i + 1) * B_LOC)
        im = {f"f{l}": feats[l][sl] for l in range(3)}
        im.update({f"mem{l}": mems[l] for l in range(3)})
        in_maps.append(im)

    res = run_bass_kernel_spmd(nc, in_maps, core_ids=list(range(NCORES)))
    kernel._last_results = res

    outs = []
    for l, (c, h, w) in enumerate(LEVELS):
        full = np.concatenate([res.results[i][f"out{l}"] for i in range(NCORES)], axis=0)
        outs.append(full.reshape(B_TOT, 2 * c, h, w))
    return tuple(outs)
